# revision 2
# baseline (speedup 1.0000x reference)
"""GCNII (8-layer GCN2Conv stack) on 8 Trainium2 NeuronCores — v3.

Strategy (1D graph parallel over destination nodes), same skeleton as v2:
  - Core c owns destination rows [c*NS, (c+1)*NS). Edges partitioned by dst,
    sorted by dst, grouped per (dst-block, src-region) into 128-edge chunks
    padded with zero-weight edges (schedule shared across cores).
  - h is exchanged per layer via two AllGathers (LO/HI region split) into a
    per-core HBM token table; per-edge source rows are fetched with
    HBM->SBUF dma_gather (plain mode, rotating SWDGE queues).
  - Per chunk: PE matmul aggT += V^T @ S accumulates [feat, dst] in PSUM.

v3 changes vs v2 (trace-driven):
  - The dense selector S (31 MB/layer of HBM loads in v2 — which halved the
    achievable gather bandwidth by competing for the same SDMA engines/HBM
    port) is now built ON-CHIP: S[e, d] = (iota[d] == dstloc[e]) * w[e] via
    one DVE tensor_scalar (is_equal, mult) per chunk from two tiny
    per-chunk streams (dstloc, w) loaded once.
  - Epilogue/prologue dense matmuls run in bf16 (4x PE rate vs f32).
"""

import numpy as np

N = 50000
E = 800000
D = 128
L = 8
ALPHA = 0.1
THETA = 0.5
NCORES = 8
P = 128

NS = N // NCORES            # 6250
NB = -(-NS // P)            # 49 blocks per core
NB_LO = 24                  # region split: blocks [0,24) -> LO
NB_HI = NB - NB_LO          # blocks [24,49) -> HI
NR_LO = NCORES * NB_LO      # 192 ranks
NR_HI = NCORES * NB_HI      # 200 ranks

_NEFF_CACHE = {}
_AG_OFF = False
_NL = L
_LAST_IN_MAPS = None


def _preprocess(edge_index, edge_weight):
    """Sort edges by dst; per core, group per (dst-block, src-region) and pad
    each group to a multiple of 128 edges (schedule shared across cores).

    Returns per-core arrays:
      dl_pm  [P, nch] bf16 : dstloc within the chunk's dst block (-1 = pad)
      w_pm   [P, nch] bf16 : edge weight * (1 - ALPHA)  (0 = pad)
      idx16  [P, nch*8] i16 : dma_gather index stream (16-row wrap, 8 stripes)
    plus per-block group spans: groups[b] = [(region, k0, m), ...]
    """
    src = np.asarray(edge_index[0], dtype=np.int64)
    dst = np.asarray(edge_index[1], dtype=np.int64)
    w = np.asarray(edge_weight, dtype=np.float32) * (1.0 - ALPHA)

    order = np.argsort(dst, kind="stable")
    src_s, dst_s, w_s = src[order], dst[order], w[order]

    core = dst_s // NS
    local = dst_s - core * NS
    blk = local // P
    dstloc = (local - blk * P).astype(np.int64)

    sc = src_s // NS
    sloc = src_s - sc * NS
    sb = sloc // P
    sp = sloc - sb * P
    hi = (sb >= NB_LO).astype(np.int64)
    tok = np.where(hi == 1,
                   (sc * P + sp) * NB_HI + (sb - NB_LO),
                   (sc * P + sp) * NB_LO + sb).astype(np.int64)

    # per (core, blk, region) counts -> shared chunks-per-group schedule
    cbg = (core * NB + blk) * 2 + hi
    counts = np.bincount(cbg, minlength=NCORES * NB * 2).reshape(NCORES, NB, 2)
    cpg = -(-counts.max(axis=0) // P)          # [NB, 2]
    cpg = np.maximum(cpg, 1)
    nch = int(cpg.sum())
    # phase-major chunk layout: all LO groups (b ascending), then all HI
    koff_lo = np.concatenate([[0], np.cumsum(cpg[:, 0])[:-1]])
    koff_hi = int(cpg[:, 0].sum()) + np.concatenate(
        [[0], np.cumsum(cpg[:, 1])[:-1]])
    group_koff = np.stack([koff_lo, koff_hi], axis=1).astype(np.int64)

    groups = []
    for b in range(NB):
        groups.append([(g, int(group_koff[b, g]), int(cpg[b, g]))
                       for g in range(2)])

    import ml_dtypes
    dl_arrs, w_arrs, idx_arrs = [], [], []
    core_starts = np.searchsorted(core, np.arange(NCORES + 1))
    for c in range(NCORES):
        s0, s1 = core_starts[c], core_starts[c + 1]
        cblk = blk[s0:s1]
        bstart = np.searchsorted(cblk, np.arange(NB + 1)) + s0
        tok16 = np.zeros(nch * P, dtype=np.int16)
        dl_flat = np.full(nch * P, -1.0, dtype=np.float32)
        w_flat = np.zeros(nch * P, dtype=np.float32)
        for b in range(NB):
            e0, e1 = bstart[b], bstart[b + 1]
            seg_hi = hi[e0:e1]
            for g in range(2):
                msk = seg_hi == g
                gtok = tok[e0:e1][msk]
                cnt = len(gtok)
                pos = group_koff[b, g] * P
                tok16[pos:pos + cnt] = gtok.astype(np.int16)
                dl_flat[pos:pos + cnt] = dstloc[e0:e1][msk]
                w_flat[pos:pos + cnt] = w_s[e0:e1][msk]
        # [P, nch]: partition = edge slot within chunk, free = chunk
        dl_pm = np.ascontiguousarray(
            dl_flat.reshape(nch, P).T).astype(ml_dtypes.bfloat16)
        w_pm = np.ascontiguousarray(w_flat.reshape(nch, P).T)
        # idx stream: 16-row wrap per gather span, replicated into 8 stripes
        ia = np.zeros((16, nch * 8), dtype=np.int16)
        for b in range(NB):
            for (g, k0, m) in groups[b]:
                sub = tok16[k0 * P:(k0 + m) * P]
                ia[:, k0 * 8:(k0 + m) * 8] = sub.reshape(m * 8, 16).T
        idx_arrs.append(np.ascontiguousarray(np.tile(ia, (8, 1))))
        dl_arrs.append(dl_pm)
        w_arrs.append(w_pm)

    return dl_arrs, w_arrs, idx_arrs, nch, groups


def _build(nc, *, nch, groups, n_layers):
    import concourse.mybir as mybir
    import concourse.tile as tile
    from concourse.masks import make_identity

    f32 = mybir.dt.float32
    bf16 = mybir.dt.bfloat16
    i16 = mybir.dt.int16

    x_t = nc.dram_tensor("x_shard", [NS, D], f32, kind="ExternalInput")
    wlin_t = nc.dram_tensor("w_lin", [D, D], f32, kind="ExternalInput")
    blin_t = nc.dram_tensor("b_lin", [D], f32, kind="ExternalInput")
    what_t = nc.dram_tensor("w_hat", [n_layers, D, D], f32,
                            kind="ExternalInput")
    dl_t = nc.dram_tensor("dl16", [P, nch], bf16, kind="ExternalInput")
    wgt_t = nc.dram_tensor("w16", [P, nch], f32, kind="ExternalInput")
    idx_t = nc.dram_tensor("idx16", [P, nch * 8], i16, kind="ExternalInput")
    out_t = nc.dram_tensor("h_out", [NS, D], f32, kind="ExternalOutput")

    # shards (partition-major: [p, b*128+f]) and gathered regions
    sh_lo = [nc.dram_tensor(f"sh_lo{i}", [P, NB_LO * D], bf16)
             for i in range(2)]
    sh_hi = [nc.dram_tensor(f"sh_hi{i}", [P, NB_HI * D], bf16)
             for i in range(2)]
    hf_lo = [nc.dram_tensor(f"hf_lo{i}", [NCORES, P, NB_LO * D], bf16,
                            addr_space="Shared") for i in range(2)]
    hf_hi = [nc.dram_tensor(f"hf_hi{i}", [NCORES, P, NB_HI * D], bf16,
                            addr_space="Shared") for i in range(2)]
    rg = [list(range(NCORES))]

    n_full = NS // P
    last = NS - n_full * P
    blk_rows = [P] * n_full + ([last] if last else [])
    assert len(blk_rows) == NB

    pairs = [list(range(b, min(b + 2, NB))) for b in range(0, NB, 2)]
    mcap = max(sum(groups[b][g][2] for b in pair)
               for g in (0, 1) for pair in pairs)

    with tile.TileContext(nc) as tc:
        with (
            tc.tile_pool(name="res", bufs=1) as res,
            tc.tile_pool(name="xp", bufs=3) as xp,
            tc.tile_pool(name="vp", bufs=7) as vp,
            tc.tile_pool(name="sp", bufs=4) as sp,
            tc.tile_pool(name="mp", bufs=4) as mp,
            tc.tile_pool(name="hp", bufs=6) as hp,
            tc.tile_pool(name="ps", bufs=2, space="PSUM") as ps,
            tc.tile_pool(name="ptr", bufs=2, space="PSUM") as ptr,
            tc.tile_pool(name="psa", bufs=3, space="PSUM") as psa,
        ):
            ident = res.tile([P, P], f32, tag="ident")
            make_identity(nc, ident[:])

            wlin_s = res.tile([P, D], f32, tag="wlin")
            nc.sync.dma_start(out=wlin_s[:], in_=wlin_t[:])
            blin_s = res.tile([P, 1], f32, tag="blin")
            nc.sync.dma_start(out=blin_s[:], in_=blin_t[:, None])
            what_s = res.tile([P, n_layers * D], f32, tag="what")
            for l in range(n_layers):
                nc.sync.dma_start(out=what_s[:, l * D:(l + 1) * D],
                                  in_=what_t[l, :, :])
            what16 = res.tile([P, n_layers * D], bf16, tag="what16")
            nc.vector.tensor_copy(out=what16[:], in_=what_s[:])
            idx16 = res.tile([P, nch * 8], i16, tag="idx16")
            nc.sync.dma_start(out=idx16[:], in_=idx_t[:])
            dl_s = res.tile([P, nch], bf16, tag="dl")
            nc.sync.dma_start(out=dl_s[:], in_=dl_t[:])
            wgt_s = res.tile([P, nch], f32, tag="wgt")
            nc.sync.dma_start(out=wgt_s[:], in_=wgt_t[:])

            # iota along the last dim (0..127 repeated mcap times)
            iota_i = res.tile([P, mcap, P], i16, tag="iotai")
            nc.gpsimd.iota(iota_i[:], pattern=[[0, mcap], [1, P]], base=0,
                           channel_multiplier=0)
            iota_b = res.tile([P, mcap, P], bf16, tag="iotab")
            nc.vector.tensor_copy(out=iota_b[:], in_=iota_i[:])

            x0sT = res.tile([P, NS], f32, tag="x0sT")  # alpha * x0.T
            stash_a = res.tile([P, NB * P], bf16, tag="stash_a")
            stash_b = res.tile([P, NB * P], bf16, tag="stash_b")
            stash2 = [stash_a, stash_b]
            hstage = res.tile([P, NB * D], bf16, tag="hstage")

            def flush_stage(idx, region):
                if region == 0:
                    nc.sync.dma_start(out=sh_lo[idx][:, :],
                                      in_=hstage[:, :NB_LO * D])
                else:
                    nc.sync.dma_start(out=sh_hi[idx][:, :],
                                      in_=hstage[:, NB_LO * D:])

            def ag(idx, region):
                sh = sh_lo if region == 0 else sh_hi
                hf = hf_lo if region == 0 else hf_hi
                if _AG_OFF:
                    nc.sync.dma_start(out=hf[idx][0, :, :], in_=sh[idx][:])
                else:
                    nc.gpsimd.collective_compute(
                        "AllGather", mybir.AluOpType.bypass,
                        replica_groups=rg, ins=[sh[idx][:]],
                        outs=[hf[idx][:]])

            # ---- prologue: x0 = relu(x @ W_lin + b) ----
            for b in range(NB):
                rows = blk_rows[b]
                r0 = b * P
                xb = xp.tile([P, D], f32, tag="xb")
                nc.sync.dma_start(out=xb[:rows, :], in_=x_t[r0:r0 + rows, :])
                xbT_ps = ptr.tile([P, P], f32, tag="tr", space="PSUM")
                nc.tensor.transpose(out=xbT_ps[:, :rows], in_=xb[:rows, :],
                                    identity=ident[:rows, :rows])
                xbT = xp.tile([P, P], f32, tag="xbT")
                nc.vector.tensor_copy(out=xbT[:, :rows], in_=xbT_ps[:, :rows])
                ps2 = ps.tile([P, P], f32, tag="dense", space="PSUM")
                nc.tensor.matmul(out=ps2[:, :rows], lhsT=wlin_s[:],
                                 rhs=xbT[:, :rows], start=True, stop=True)
                x0Tb = xp.tile([P, P], f32, tag="x0Tb")
                nc.scalar.activation(out=x0Tb[:, :rows], in_=ps2[:, :rows],
                                     func=mybir.ActivationFunctionType.Relu,
                                     bias=blin_s[:, :1], scale=1.0)
                nc.vector.tensor_scalar(
                    out=x0sT[:, r0:r0 + rows], in0=x0Tb[:, :rows],
                    scalar1=ALPHA, scalar2=None, op0=mybir.AluOpType.mult)
                x0_ps = ptr.tile([P, P], f32, tag="tr", space="PSUM")
                nc.tensor.transpose(out=x0_ps[:rows, :], in_=x0Tb[:, :rows],
                                    identity=ident[:])
                nc.vector.tensor_copy(
                    out=hstage[:rows, b * D:(b + 1) * D], in_=x0_ps[:rows, :])
                if b == NB_LO - 1:
                    flush_stage(0, 0)
                    ag(0, 0)
            flush_stage(0, 1)
            ag(0, 1)

            # ---- layers ----
            gq = 0

            def epilogue(l, b, aggT, is_last, nxt):
                rows = blk_rows[b]
                r0 = b * P
                mT = mp.tile([P, P], bf16, tag="mT")
                nc.vector.tensor_tensor(
                    out=mT[:, :rows], in0=aggT[:, :rows],
                    in1=x0sT[:, r0:r0 + rows], op=mybir.AluOpType.add)
                w_ap = what16[:, l * D:(l + 1) * D]
                ps2 = ps.tile([P, P], f32, tag="dense", space="PSUM")
                nc.tensor.matmul(out=ps2[:rows, :], lhsT=mT[:, :rows],
                                 rhs=w_ap, start=True, stop=True)
                if is_last:
                    hb = hp.tile([P, D], f32, tag="hbo")
                    nc.scalar.activation(
                        out=hb[:rows, :], in_=ps2[:rows, :],
                        func=mybir.ActivationFunctionType.Relu)
                    nc.sync.dma_start(out=out_t[r0:r0 + rows, :],
                                      in_=hb[:rows, :])
                else:
                    nc.scalar.activation(
                        out=hstage[:rows, b * D:(b + 1) * D],
                        in_=ps2[:rows, :],
                        func=mybir.ActivationFunctionType.Relu)
                    if b == NB_LO - 1:
                        flush_stage(nxt, 0)
                        ag(nxt, 0)

            def do_phase(cur, g_sel, pair, blk_cb):
                nonlocal gq
                stash = stash2[cur]
                k0 = groups[pair[0]][g_sel][1]
                mspan = sum(groups[b][g_sel][2] for b in pair)
                # on-chip selector build: S[e, k, d] = (iota[d]==dl[e,k])*w[e,k]
                # onehot on DVE (one batched op), then per-chunk scale by w
                # on the Scalar engine (w[:, k] is a per-partition scalar)
                s_blk = sp.tile([P, mcap, P], bf16, tag="s")
                import concourse.bass as bass
                _, dlb = bass.broadcast_tensor_aps(
                    s_blk[:, :mspan, :], dl_s[:, k0:k0 + mspan, None])
                nc.vector.tensor_tensor(
                    out=s_blk[:, :mspan, :], in0=iota_b[:, :mspan, :],
                    in1=dlb, op=mybir.AluOpType.is_equal)
                for j in range(mspan):
                    nc.scalar.activation(
                        out=s_blk[:, j, :], in_=s_blk[:, j, :],
                        func=mybir.ActivationFunctionType.Copy,
                        scale=wgt_s[:, k0 + j:k0 + j + 1])
                hf = hf_lo if g_sel == 0 else hf_hi
                vg = vp.tile([P, mcap, D], bf16, tag="v")
                nc.gpsimd.dma_gather(
                    vg[:, :mspan, :],
                    hf[cur].rearrange("c p (b f) -> (c p b) f", f=D),
                    idx16[:, k0 * 8:(k0 + mspan) * 8],
                    num_idxs=mspan * P, num_idxs_reg=mspan * P,
                    elem_size=D, single_packet=False,
                    queue_num=gq % 4)
                gq += 1
                krel = 0
                for b in pair:
                    m = groups[b][g_sel][2]
                    aggT = psa.tile([P, P], f32, tag="agg", space="PSUM")
                    open_group = g_sel == 0
                    if not open_group:
                        nc.vector.tensor_copy(
                            out=aggT[:], in_=stash[:, b * P:(b + 1) * P])
                    for k in range(m):
                        nc.tensor.matmul(
                            out=aggT[:], lhsT=vg[:, krel + k, :],
                            rhs=s_blk[:, krel + k, :],
                            start=(open_group and k == 0), stop=(k == m - 1),
                            skip_group_check=not open_group)
                    krel += m
                    blk_cb(b, aggT)

            for l in range(n_layers):
                cur = l % 2
                nxt = (l + 1) % 2
                is_last = l == n_layers - 1

                def stash_cb(b, aggT, cur=cur):
                    nc.vector.tensor_copy(
                        out=stash2[cur][:, b * P:(b + 1) * P], in_=aggT[:])

                def epi_cb(b, aggT, l=l, is_last=is_last, nxt=nxt):
                    epilogue(l, b, aggT, is_last, nxt)

                for pair in pairs:
                    do_phase(cur, 0, pair, stash_cb)
                for pair in pairs:
                    do_phase(cur, 1, pair, epi_cb)
                if not is_last:
                    flush_stage(nxt, 1)
                    ag(nxt, 1)
    return nc


def _run(inputs, *, n_layers=L, ncores=NCORES):
    import concourse.bacc as bacc
    from concourse.bass_utils import run_bass_kernel_spmd

    x = np.asarray(inputs["x"], dtype=np.float32)
    edge_weight = np.asarray(inputs["edge_weight"], dtype=np.float32)
    w_lin = np.asarray(inputs["W_lin"], dtype=np.float32)
    b_lin = np.asarray(inputs["b_lin"], dtype=np.float32)
    w_convs = np.asarray(inputs["W_convs"], dtype=np.float32)
    edge_index = np.asarray(inputs["edge_index"])

    betas = np.log(THETA / np.arange(1, n_layers + 1) + 1.0).astype(np.float32)
    eye = np.eye(D, dtype=np.float32)
    w_hat = np.stack([betas[l] * w_convs[l] + (1.0 - betas[l]) * eye
                      for l in range(n_layers)]).astype(np.float32)

    dl_arrs, w_arrs, idx_arrs, nch, groups = _preprocess(
        edge_index, edge_weight)

    key = (n_layers, nch, tuple(g[2] for b in groups for g in b))
    if key not in _NEFF_CACHE:
        nc = bacc.Bacc("TRN2", target_bir_lowering=False, debug=False,
                       num_devices=ncores, num_swdge_queues=4)
        _build(nc, nch=nch, groups=groups, n_layers=n_layers)
        nc.compile()
        _NEFF_CACHE[key] = nc
    nc = _NEFF_CACHE[key]

    in_maps = []
    for c in range(ncores):
        in_maps.append({
            "x_shard": np.ascontiguousarray(x[c * NS:(c + 1) * NS]),
            "w_lin": w_lin, "b_lin": b_lin, "w_hat": w_hat,
            "dl16": dl_arrs[c], "w16": w_arrs[c], "idx16": idx_arrs[c],
        })
    global _LAST_IN_MAPS
    _LAST_IN_MAPS = in_maps
    res = run_bass_kernel_spmd(nc, in_maps, list(range(ncores)))
    out = np.concatenate([res.results[c]["h_out"] for c in range(ncores)],
                         axis=0)
    return out


def kernel(x, edge_weight, W_lin, b_lin, W_convs, edge_index):
    return _run(dict(x=x, edge_weight=edge_weight, W_lin=W_lin, b_lin=b_lin,
                     W_convs=W_convs, edge_index=edge_index), n_layers=_NL)
